# revision 1
# baseline (speedup 1.0000x reference)
"""LoRA Multihead Attention on 8 TRN2 NeuronCores.

Sharding: tensor-parallel attention over heads, token-parallel epilogue.
Core c owns heads {2c, 2c+1} (= channel slice [128c, 128c+128)) for the
projections + attention, and owns tokens l in [256c, 256c+256) of each
batch for the out_proj/LoRA epilogue. Each core:
  1. computes q,k (feature-major) and v (token-major) projections for its
     heads,
  2. runs attention S^T = k^T q (row-tiled: both heads concurrently in the
     PE array), P = exp(S^T) in 1024-wide Act instructions, P@V col-tiled
     (both heads concurrently), softmax denominators via 4-way col-tiled
     ones-matmuls (normalization deferred past the P@V accumulation),
  3. AllToAll per batch: cores exchange 256-token x 128-channel blocks so
     every core ends with all 1024 channels of its own 256-token slice
     (8x less traffic than the AllGather of the full activation),
  4. computes the FULL out_proj + LoRA for its 512 tokens (both batches).
Host reassembles the 8 token slices and restores (L, N, E) layout.

The emission order interleaves projection / epilogue matmul chunks into
the attention j-loop so the PE stays busy during the Act engine's exp
instructions (which otherwise serialize the pipeline), and the exp
table is pre-loaded during the input DMA phase.

All matmuls bf16 with fp32 PSUM accumulation; softmax statistics in fp32.
"""

import os
import sys
from collections import deque

sys.path.insert(0, "/opt/trn_rl_repo")

import numpy as np
import ml_dtypes

import concourse.bass as bass  # noqa: F401  (import keeps bass registered)
import concourse.tile as tile
from concourse import bacc, mybir
from concourse.bass_utils import run_bass_kernel_spmd

BF = ml_dtypes.bfloat16
bf16 = mybir.dt.bfloat16
f32 = mybir.dt.float32

L, N, E = 2048, 2, 1024
T = N * L            # 4096 tokens, t = n*L + l
H, D, R = 16, 64, 16
NCORES = 8
HPC = H // NCORES    # heads per core = 2
CS = HPC * D         # channel slice width per core = 128
TPC = T // NCORES    # tokens per core in the epilogue = 512
LPC = L // NCORES    # l-slice per core per batch = 256
SCALE = D ** -0.5
LORA_SCALING = 32.0 / 16.0

LB = 512             # l-block (moving free dim)
NT = T // LB         # 8 t-blocks over all tokens
NTB = NT // N        # 4 t-blocks per batch
NLB = L // LB        # 4 l-blocks per batch
NMT = L // 128       # 16 m-tiles per batch
NJ = NMT // 2        # 8 m-tile pairs per batch
NE = E // 128        # 8 contraction tiles

_CACHE = {}

K_NOBCAST = bool(int(os.environ.get("K_NOBCAST", "0")))
K_SYNCSHIP = bool(int(os.environ.get("K_SYNCSHIP", "0")))
K_NOEXP = bool(int(os.environ.get("K_NOEXP", "0")))
K_NONORM = bool(int(os.environ.get("K_NONORM", "0")))


def _build_nc(reps=1, stages=("proj", "attn", "ag", "outproj")):
    nc = bacc.Bacc("TRN2", target_bir_lowering=False, debug=False,
                   enable_asserts=False, num_devices=NCORES)

    qT_d = nc.dram_tensor("qT", [E, T], bf16, kind="ExternalInput")
    wqkt_d = nc.dram_tensor("wqkt", [E, 2 * CS], bf16, kind="ExternalInput")
    wvt_d = nc.dram_tensor("wvt", [E, CS], bf16, kind="ExternalInput")
    bqk_d = nc.dram_tensor("bqk", [2 * CS, 1], f32, kind="ExternalInput")
    woutt_d = nc.dram_tensor("woutt", [E, E], bf16, kind="ExternalInput")
    at_d = nc.dram_tensor("at", [E, R], bf16, kind="ExternalInput")
    btf_d = nc.dram_tensor("btf", [R, E], bf16, kind="ExternalInput")
    bout_d = nc.dram_tensor("bout", [E, 1], f32, kind="ExternalInput")
    outp_d = nc.dram_tensor("outp", [E, TPC], f32, kind="ExternalOutput")

    a2a_in = [nc.dram_tensor(f"a2a_in{n}", [E, LPC], bf16) for n in range(N)]
    a2a_out = [nc.dram_tensor(f"a2a_out{n}", [E, LPC], bf16) for n in range(N)]

    with tile.TileContext(nc) as tc:
        with (
            tc.tile_pool(name="const", bufs=1) as cp,
            tc.tile_pool(name="qt", bufs=1) as qtp,
            tc.tile_pool(name="qks", bufs=1) as qksp,
            tc.tile_pool(name="vp", bufs=1) as vp,
            tc.tile_pool(name="pp", bufs=4) as pp,
            tc.tile_pool(name="osb", bufs=1) as osbp,
            tc.tile_pool(name="ot", bufs=16) as otp,
            tc.tile_pool(name="small", bufs=4) as smp,
            tc.tile_pool(name="rr", bufs=2) as rrp,
            tc.tile_pool(name="ob", bufs=3) as obp,
            tc.tile_pool(name="ps_s", bufs=2, space="PSUM") as ps_s,
            tc.tile_pool(name="ps_o", bufs=1, space="PSUM") as ps_o,
            tc.tile_pool(name="ps_d", bufs=1, space="PSUM") as ps_d,
            tc.tile_pool(name="ps_m", bufs=2, space="PSUM") as ps_m,
        ):
            # ---- constants & inputs; batch-0/tb0 query columns first ----
            wqkt = [cp.tile([128, 2 * CS], bf16, tag=f"wqkt{e}", name=f"wqkt{e}") for e in range(NE)]
            wvt = [cp.tile([128, CS], bf16, tag=f"wvt{e}", name=f"wvt{e}") for e in range(NE)]
            woutt = [cp.tile([128, E], bf16, tag=f"woutt{e}", name=f"woutt{e}") for e in range(NE)]
            at = [cp.tile([128, R], bf16, tag=f"at{e}", name=f"at{e}") for e in range(NE)]
            btf = cp.tile([R, E], bf16, tag="btf", name="btf")
            bqk = [cp.tile([128, 1], f32, tag=f"bqk{ch}", name=f"bqk{ch}") for ch in range(2)]
            bout = [cp.tile([128, 1], f32, tag=f"bout{e}", name=f"bout{e}") for e in range(NE)]
            ones = cp.tile([128, 1], bf16, tag="ones", name="ones")
            nc.vector.memset(ones[:], 1.0)
            # pre-load the exp spline tables while input DMAs run
            warm = cp.tile([1, 8], f32, tag="warm", name="warm")
            nc.vector.memset(warm[:], 0.0)
            nc.scalar.activation(warm[:], warm[:], mybir.ActivationFunctionType.Exp)

            qt = [qtp.tile([128, T], bf16, tag=f"qt{e}", name=f"qt{e}") for e in range(NE)]
            for e in range(NE):
                sl = slice(e * 128, (e + 1) * 128)
                nc.sync.dma_start(qt[e][:, 0:LB], qT_d.ap()[sl, 0:LB])
                nc.sync.dma_start(wqkt[e][:], wqkt_d.ap()[sl, :])
            nc.sync.dma_start(bqk[0][:], bqk_d.ap()[0:CS, :])
            nc.sync.dma_start(bqk[1][:], bqk_d.ap()[CS:2 * CS, :])
            for e in range(NE):
                sl = slice(e * 128, (e + 1) * 128)
                nc.sync.dma_start(qt[e][:, LB:L], qT_d.ap()[sl, LB:L])
                nc.sync.dma_start(wvt[e][:], wvt_d.ap()[sl, :])
            for e in range(NE):
                sl = slice(e * 128, (e + 1) * 128)
                nc.sync.dma_start(qt[e][:, L:T], qT_d.ap()[sl, L:T])
                nc.sync.dma_start(woutt[e][:], woutt_d.ap()[sl, :])
                nc.sync.dma_start(at[e][:], at_d.ap()[sl, :])
                nc.sync.dma_start(bout[e][:], bout_d.ap()[sl, :])
            nc.sync.dma_start(btf[:], btf_d.ap())

            for _rep in range(reps):
              qks = [qksp.tile([128, T], bf16, tag=f"qks{ch}", name=f"qks{ch}") for ch in range(2)]
              v_all = [[vp.tile([128, NMT * D], bf16, tag=f"v{n}{h}", name=f"v{n}{h}")
                        for h in range(2)] for n in range(N)]
              osb = [osbp.tile([CS, L], bf16, tag=f"osb{n}", name=f"osb{n}")
                     for n in range(N)]

              # ---- emission units ----
              def emit_qk(n, t):
                  """q,k projection for 512-token block t of batch n."""
                  tb = n * NTB + t
                  cs = slice(tb * LB, (tb + 1) * LB)
                  for ch in range(2):
                      pm = ps_m.tile([128, LB], f32, tag="m", name="pm")
                      for e in range(NE):
                          nc.tensor.matmul(pm[:], wqkt[e][:, ch * CS:(ch + 1) * CS],
                                           qt[e][:, cs], start=(e == 0), stop=(e == NE - 1))
                      nc.vector.tensor_scalar_add(qks[ch][:, cs], pm[:], bqk[ch][:])

              def emit_v(n, g):
                  """v projection (token-major) for m-tiles 4g..4g+3 of batch n."""
                  for mti in range(4 * g, 4 * g + 4):
                      mt = n * NMT + mti
                      pm = ps_m.tile([128, CS], f32, tag="m", name="pmv")
                      cs = slice(mt * 128, (mt + 1) * 128)
                      for e in range(NE):
                          nc.tensor.matmul(pm[:], qt[e][:, cs], wvt[e][:],
                                           start=(e == 0), stop=(e == NE - 1))
                      for h in range(2):
                          nc.vector.tensor_copy(
                              v_all[n][h][:, mti * D:(mti + 1) * D],
                              pm[:, h * D:(h + 1) * D])

              ep_state = {}

              def emit_ep_pre(n):
                  """epilogue preamble: fetch this core's token block, LoRA r."""
                  ot = []
                  for e in range(NE):
                      t_ = otp.tile([128, LPC], bf16, tag="ot", name="ott")
                      nc.sync.dma_start(t_[:], a2a_out[n].ap()[e * 128:(e + 1) * 128, :])
                      ot.append(t_)
                  rt_ps = ps_m.tile([128, LPC], f32, tag="m", name="rtps")
                  for e in range(NE):
                      s = e % 4
                      nc.tensor.matmul(rt_ps[32 * s:32 * s + R, :], at[e][:], ot[e][:],
                                       start=(e < 4), stop=(e >= 4),
                                       tile_position=(0, 32 * s),
                                       skip_group_check=True)
                  ra0 = smp.tile([R, LPC], f32, tag="ra0", name="ra0")
                  ra1 = smp.tile([R, LPC], f32, tag="ra1", name="ra1")
                  nc.vector.tensor_copy(ra0[:], rt_ps[0:R, :])
                  nc.vector.tensor_copy(ra1[:], rt_ps[64:64 + R, :])
                  r0 = smp.tile([R, LPC], f32, tag="r0", name="r0")
                  r1 = smp.tile([R, LPC], f32, tag="r1", name="r1")
                  nc.vector.tensor_add(r0[:], ra0[:], rt_ps[32:32 + R, :])
                  nc.vector.tensor_add(r1[:], ra1[:], rt_ps[96:96 + R, :])
                  rt_sb = smp.tile([R, LPC], bf16, tag="rt", name="rtsb")
                  nc.vector.tensor_add(rt_sb[:], r0[:], r1[:])
                  ep_state[n] = (ot, rt_sb)

              def emit_ep_eo(n, eo):
                  """epilogue: output row-block eo for batch-n token slice."""
                  ot, rt_sb = ep_state[n]
                  f_ps = ps_m.tile([128, LPC], f32, tag="m", name="fps")
                  eos = slice(eo * 128, (eo + 1) * 128)
                  for e in range(NE):
                      nc.tensor.matmul(f_ps[:], woutt[e][:, eos], ot[e][:],
                                       start=(e == 0), stop=False)
                  nc.tensor.matmul(f_ps[:], btf[:, eos], rt_sb[:],
                                   start=False, stop=True)
                  ob = obp.tile([128, LPC], f32, tag="ob", name="obt")
                  nc.vector.tensor_scalar_add(ob[:], f_ps[:], bout[eo][:])
                  nc.sync.dma_start(outp_d.ap()[eos, n * LPC:(n + 1) * LPC], ob[:])

              fillers = deque()

              def pop_filler(k=1):
                  for _ in range(k):
                      if fillers:
                          fillers.popleft()()

              # ---- interleaved schedule ----
              do_attn = "attn" in stages
              do_ag = "ag" in stages
              do_ep = "outproj" in stages

              emit_qk(0, 0)
              emit_v(0, 0)
              for n in range(N):
                  base = n * L
                  for lb in range(NLB):
                      ls = slice(base + lb * LB, base + (lb + 1) * LB)
                      lsl = slice(lb * LB, (lb + 1) * LB)
                      if not do_attn:
                          if n == 0 and lb == 0:
                              for t in range(1, NTB):
                                  emit_qk(0, t)
                                  emit_v(0, t)
                              for t in range(NTB):
                                  emit_qk(1, t)
                                  emit_v(1, t)
                          nc.vector.memset(osb[n][:, lsl], 0.5)
                      o_ps = d_ps = None
                      for j in range(NJ) if do_attn else []:
                          if n == 0 and lb == 0 and j in (2, 4, 6):
                              emit_qk(0, j // 2)
                          if j == 0:
                              o_ps = ps_o.tile([128, LB], f32, tag="acc", name="ops")
                              d_ps = ps_d.tile([128, LB], f32, tag="den", name="dps")
                          # S^T pair: heads row-tiled, two m-tiles per bank-pair
                          s_ps = [ps_s.tile([128, 2 * LB], f32, tag="s", name="sps")
                                  for _ in range(2)]
                          for t in range(2):
                              ms = slice(base + (2 * j + t) * 128,
                                         base + (2 * j + t + 1) * 128)
                              for h in range(2):
                                  d0 = h * D
                                  nc.tensor.matmul(s_ps[h][:, t * LB:(t + 1) * LB],
                                                   qks[1][d0:d0 + D, ms],
                                                   qks[0][d0:d0 + D, ls],
                                                   start=True, stop=True)
                          p_t = []
                          for h in range(2):
                              pt = pp.tile([128, 2 * LB], bf16, tag="p", name="pt")
                              if K_NOEXP:
                                  nc.vector.tensor_copy(pt[:], s_ps[h][:])
                              else:
                                  nc.scalar.activation(pt[:], s_ps[h][:],
                                                       mybir.ActivationFunctionType.Exp)
                              p_t.append(pt)
                          # PE filler under the exp latency
                          if n == 0 and lb == 0:
                              if j in (2, 4, 6):
                                  emit_v(0, j // 2)
                          else:
                              pop_filler(1)
                          # P@V: heads col-tiled into one PSUM bank
                          for t in range(2):
                              mti = 2 * j + t
                              for h in range(2):
                                  nc.tensor.matmul(o_ps[h * D:(h + 1) * D, :],
                                                   v_all[n][h][:, mti * D:(mti + 1) * D],
                                                   p_t[h][:, t * LB:(t + 1) * LB],
                                                   start=(j == 0 and t == 0),
                                                   stop=(j == NJ - 1 and t == 1),
                                                   skip_group_check=True)
                          # denominators: 4-way col-tiled ones-matmuls
                          for h in range(2):
                              for t in range(2):
                                  s = 2 * h + t
                                  nc.tensor.matmul(d_ps[32 * s:32 * s + 1, :],
                                                   ones[:],
                                                   p_t[h][:, t * LB:(t + 1) * LB],
                                                   start=(j == 0), stop=(j == NJ - 1),
                                                   tile_position=(0, 32 * s),
                                                   skip_group_check=True)
                      # normalization: per head reciprocal of strip sums,
                      # broadcast, multiply into osb
                      for h in range(2) if do_attn else []:
                          ra = smp.tile([1, LB], f32, tag=f"ra{h}", name=f"ra{h}")
                          nc.vector.tensor_copy(ra[:], d_ps[64 * h:64 * h + 1, :])
                          rs = smp.tile([1, LB], f32, tag=f"rs{h}", name=f"rs{h}")
                          nc.vector.tensor_add(rs[:], ra[:],
                                               d_ps[64 * h + 32:64 * h + 33, :])
                          rc = smp.tile([1, LB], f32, tag=f"rc{h}", name=f"rc{h}")
                          nc.vector.reciprocal(rc[:], rs[:])
                          rr1 = rrp.tile([D, LB], f32, tag=f"rr{h}", name="rr1")
                          if not K_NOBCAST:
                              nc.gpsimd.partition_broadcast(rr1[:], rc[:])
                          nc.vector.tensor_mul(osb[n][h * D:(h + 1) * D, lsl],
                                               o_ps[h * D:(h + 1) * D, :], rr1[:])
                      # ship the two 256-token peer blocks of this l-block
                      if do_ag:
                          for jj in (2 * lb, 2 * lb + 1):
                              (nc.sync.dma_start if K_SYNCSHIP else nc.gpsimd.dma_start)(
                                  a2a_in[n].ap()[CS * jj:CS * (jj + 1), :],
                                  osb[n][:, LPC * jj:LPC * (jj + 1)])
                      # enqueue batch-1 projections as fillers for the
                      # remaining batch-0 attention
                      if n == 0 and lb == 0:
                          for t in range(NTB):
                              fillers.append(lambda t=t: emit_qk(1, t))
                              fillers.append(lambda t=t: emit_v(1, t))
                  # end of batch: drain pending projections, launch exchange
                  if n == 0:
                      pop_filler(len(fillers))
                  if do_ag:
                      nc.gpsimd.collective_compute(
                          "AllToAll", mybir.AluOpType.bypass,
                          ins=[a2a_in[n].ap()], outs=[a2a_out[n].ap()],
                          replica_groups=[list(range(NCORES))],
                      )
                  if do_ep and n == 0:
                      fillers.append(lambda: emit_ep_pre(0))
                      for eo in range(NE):
                          fillers.append(lambda eo=eo: emit_ep_eo(0, eo))
              # tail: finish batch-0 epilogue, then batch-1 epilogue
              if do_ep:
                  pop_filler(len(fillers))
                  emit_ep_pre(1)
                  for eo in range(NE):
                      emit_ep_eo(1, eo)

    nc.compile()
    return nc


def _host_prep(inputs):
    q = np.asarray(inputs["query"], np.float32)
    W = np.asarray(inputs["in_proj_weight"], np.float32)
    b = np.asarray(inputs["in_proj_bias"], np.float32)
    Wout = np.asarray(inputs["out_proj_weight"], np.float32)
    bout = np.asarray(inputs["out_proj_bias"], np.float32)
    A = np.asarray(inputs["lora_A"], np.float32)
    B = np.asarray(inputs["lora_B"], np.float32)

    qT = np.ascontiguousarray(q.transpose(2, 1, 0).reshape(E, T)).astype(BF)
    bv = b[2 * E:3 * E]
    bout_eff = bout + Wout @ bv + LORA_SCALING * (B @ (A @ bv))
    AT = np.ascontiguousarray(A.T).astype(BF)
    wouttF = np.ascontiguousarray(Wout.T).astype(BF)          # (E, E)
    btfF = np.ascontiguousarray((B * LORA_SCALING).T).astype(BF)  # (R, E)
    boutF = np.ascontiguousarray(bout_eff[:, None], np.float32)

    in_maps = []
    for c in range(NCORES):
        hs = slice(CS * c, CS * (c + 1))
        wq = W[hs, :] * SCALE
        wk = W[E + CS * c:E + CS * (c + 1), :]
        wv = W[2 * E + CS * c:2 * E + CS * (c + 1), :]
        wqkt = np.ascontiguousarray(np.concatenate([wq.T, wk.T], axis=1)).astype(BF)
        wvt = np.ascontiguousarray(wv.T).astype(BF)
        bqk = np.concatenate([b[hs] * SCALE, b[E + CS * c:E + CS * (c + 1)]])
        in_maps.append({
            "qT": qT,
            "wqkt": wqkt,
            "wvt": wvt,
            "bqk": np.ascontiguousarray(bqk[:, None], np.float32),
            "woutt": wouttF,
            "at": AT,
            "btf": btfF,
            "bout": boutF,
        })
    return in_maps


def _run(inputs, trace=False):
    if "nc" not in _CACHE:
        _CACHE["nc"] = _build_nc()
    nc = _CACHE["nc"]
    in_maps = _host_prep(inputs)
    res = run_bass_kernel_spmd(nc, in_maps, core_ids=list(range(NCORES)),
                               trace=trace)
    # core c holds all E channels for tokens {(n, l): l in [256c, 256c+256)}
    full = np.empty((E, N, L), np.float32)
    for c in range(NCORES):
        o = res.results[c]["outp"]                  # (E, 512)
        for n in range(N):
            full[:, n, LPC * c:LPC * (c + 1)] = o[:, n * LPC:(n + 1) * LPC]
    out = np.ascontiguousarray(full.transpose(2, 1, 0))
    return out, res


def kernel(**inputs):
    out, _ = _run(inputs, trace=False)
    return out

